# revision 14
# baseline (speedup 1.0000x reference)
"""Trainium2 Bass kernel for nn_DecoderBlock (self-attn + cross-attn + MLP).

Sharding: pure data-parallel over batch (B=8 -> 8 NeuronCores), no
collectives. Per core the whole block runs with activations feature-major
(features on SBUF partitions, tokens on the free axis) so every matmul uses
the natural [in, out] weight layout as lhsT.

vs. the earlier version: the residual stream and activations are bf16
end-to-end (inputs pre-cast on host, output cast back); RoPE's rotate-half
is a cheap K=128 permutation matmul (block +-1 matrix) applied to the
biased Q/K projections instead of a second K=768 projection with permuted
weights; LayerNorm stats are applied with bf16 2x-mode vector ops; softmax
runs unnormalized with the denominator folded into the V lhsT as a ones
column, reciprocal'd via ACT Ln -> Exp(-x) (same act table as the softmax
Exp) and partition-broadcast with a DMA bounce through DRAM; scores/exp/AV
are fused per key-chunk with double-buffered PSUM; x/y load per-chunk and
the MLP stores the output per-chunk for head/tail overlap.
"""
import numpy as np

DIM, HEADS, HD = 768, 12, 64
N = 1024
B = 8
EPS = 1e-5
FREQ = 100.0
P = 128
C = DIM // P            # 6 feature chunks
NP = HEADS // 2         # 6 head pairs
KC = N // P             # 8 key chunks
T = N // P              # 8 token chunks
HM = (4 * DIM) // P     # 24 hidden chunks


# ---------------------------------------------------------------- host prep

def _rope_tables(pos2d):
    """pos2d [N,2] int -> cos, sin [64, N] fp32 (y-half then x-half)."""
    j = np.arange(16, dtype=np.float32)
    inv = 1.0 / (FREQ ** (2.0 * j / 32.0))
    n = pos2d.shape[0]
    c = np.empty((64, n), np.float32)
    s = np.empty((64, n), np.float32)
    for half, p in ((0, pos2d[:, 0]), (1, pos2d[:, 1])):
        f = p.astype(np.float32)[None, :] * inv[:, None]
        emb = np.concatenate([f, f], 0)
        c[half * 32:(half + 1) * 32] = np.cos(emb)
        s[half * 32:(half + 1) * 32] = np.sin(emb)
    return c, s


def _rot_mat():
    """[128,128] permutation: (Prot.T @ u)[m] = rotate-half(u)[m]."""
    pm = np.zeros((P, P), np.float32)
    for m in range(P):
        if m % 32 < 16:
            pm[m + 16, m] = -1.0
        else:
            pm[m - 16, m] = 1.0
    return pm


def _feat_major(b):
    return np.ascontiguousarray(b.reshape(-1, P).T.astype(np.float32))


def prep_host(inputs):
    """Returns a list of per-core input dicts (weights shared)."""
    import ml_dtypes
    f32 = np.float32
    bf = ml_dtypes.bfloat16
    inp = {k: np.asarray(v) for k, v in inputs.items()}
    g1, b1 = inp['norm1_g'].astype(f32), inp['norm1_b'].astype(f32)
    g2, b2 = inp['norm2_g'].astype(f32), inp['norm2_b'].astype(f32)
    g3, b3 = inp['norm3_g'].astype(f32), inp['norm3_b'].astype(f32)
    gy, by = inp['normy_g'].astype(f32), inp['normy_b'].astype(f32)
    qkv = inp['qkv_w'].astype(f32)
    wq, wk, wv = qkv[:, :DIM], qkv[:, DIM:2 * DIM], qkv[:, 2 * DIM:]

    def fold(g, b, wmat):
        return (g[:, None] * wmat).astype(f32), (b @ wmat).astype(f32)

    wqA, bq = fold(g1, b1, wq)
    wkA, bk = fold(g1, b1, wk)
    wvF, bv = fold(g1, b1, wv)
    wcqA, bcq = fold(g2, b2, inp['projq_w'].astype(f32))
    wckA, bck = fold(gy, by, inp['projk_w'].astype(f32))
    wcvF, bcv = fold(gy, by, inp['projv_w'].astype(f32))
    wfc1, bfc1x = fold(g3, b3, inp['fc1_w'].astype(f32))
    bfc1 = inp['fc1_b'].astype(f32) + bfc1x

    def tiled(w):
        rows, cols = w.shape
        cr, ncb = rows // P, cols // P
        return np.ascontiguousarray(
            np.transpose(w.reshape(cr, P, ncb, P), (2, 1, 0, 3)).astype(bf))

    def vfull(w):
        cr = w.shape[0] // P
        return np.ascontiguousarray(
            np.transpose(w.reshape(cr, P, w.shape[1]), (1, 0, 2)).astype(bf))

    shared = {
        'wqA': tiled(wqA), 'wkA': tiled(wkA), 'wv': vfull(wvF),
        'wproj': tiled(inp['attn_proj_w'].astype(f32)),
        'wcqA': tiled(wcqA), 'wckA': tiled(wckA), 'wcv': vfull(wcvF),
        'wcproj': tiled(inp['cross_proj_w'].astype(f32)),
        'wfc1': tiled(wfc1), 'wfc2': tiled(inp['fc2_w'].astype(f32)),
        'bq': _feat_major(bq), 'bk': _feat_major(bk),
        'bcq': _feat_major(bcq), 'bck': _feat_major(bck),
        'bproj': _feat_major(inp['attn_proj_b'].astype(f32)),
        'bcproj': _feat_major(inp['cross_proj_b'].astype(f32)),
        'bfc1': np.ascontiguousarray(bfc1.reshape(-1, P).T.astype(f32)),
        'bfc2': _feat_major(inp['fc2_b'].astype(f32)),
        'bv_row': bv.reshape(1, DIM).astype(bf),
        'bcv_row': bcv.reshape(1, DIM).astype(bf),
        'ones_bf': np.ones((P, P), bf),
        'prot': _rot_mat().astype(bf),
    }
    per_core = []
    for bi in range(B):
        cxx, sxn = _rope_tables(inp['xpos'][bi])
        cyn, syn = _rope_tables(inp['ypos'][bi])
        d = {
            'xT': np.ascontiguousarray(inp['x'][bi].T.astype(bf)),
            'yT': np.ascontiguousarray(inp['y'][bi].T.astype(bf)),
            'cosx': np.ascontiguousarray(np.tile(cxx, (2, 1)).astype(bf)),
            'sinx': np.ascontiguousarray(np.tile(sxn, (2, 1)).astype(bf)),
            'cosy': np.ascontiguousarray(np.tile(cyn, (2, 1)).astype(bf)),
            'siny': np.ascontiguousarray(np.tile(syn, (2, 1)).astype(bf)),
        }
        d.update(shared)
        per_core.append(d)
    return per_core


# ------------------------------------------------------- walrus workarounds

def split_excess_waits(nc, max_waits=1):
    """This walrus build rejects instructions carrying more than one
    sync-wait on CTRL-class instructions. Move excess waits onto NoOps
    inserted immediately before the offending instruction on the same
    engine (same-engine program order keeps semantics)."""
    import concourse.mybir as mybir
    n_split = 0
    cnt = [0]
    for f in nc.m.functions:
        for blk in f.blocks:
            insts = list(blk.instructions)
            out = []
            changed = False
            for inst in insts:
                si = inst.sync_info
                waits = list(si.on_wait) if si and si.on_wait else []
                if len(waits) > max_waits:
                    changed = True
                    n_split += 1
                    extra = waits[:-max_waits]
                    keep = waits[-max_waits:]
                    while extra:
                        chunk, extra = extra[:max_waits], extra[max_waits:]
                        cnt[0] += 1
                        nop = mybir.InstNoOp(
                            name=f"WSPLIT-{id(nc) % 100000}-{cnt[0]}",
                            ins=[], outs=[], engine=inst.engine)
                        nop.sync_info = mybir.SyncInfo(on_wait=chunk,
                                                       on_update=[])
                        out.append(nop)
                    inst.sync_info = mybir.SyncInfo(
                        on_wait=keep,
                        on_update=list(si.on_update) if si.on_update else [])
                out.append(inst)
            if changed:
                blk.instructions = out
    return n_split


# ------------------------------------------------------------- kernel build

def build_nc(k_iters=1):
    import concourse.bass as bass
    import concourse.mybir as mybir
    from concourse.tile import TileContext

    F32 = mybir.dt.float32
    BF16 = mybir.dt.bfloat16
    AF = mybir.ActivationFunctionType
    OP = mybir.AluOpType

    nc = bass.Bass()
    d = {}
    for name, shape, dt in [
        ('xT', [DIM, N], BF16), ('yT', [DIM, N], BF16),
        ('cosx', [P, N], BF16), ('sinx', [P, N], BF16),
        ('cosy', [P, N], BF16), ('siny', [P, N], BF16),
        ('wqA', [C, P, C, P], BF16), ('wkA', [C, P, C, P], BF16),
        ('wv', [P, C, DIM], BF16), ('wproj', [C, P, C, P], BF16),
        ('wcqA', [C, P, C, P], BF16), ('wckA', [C, P, C, P], BF16),
        ('wcv', [P, C, DIM], BF16), ('wcproj', [C, P, C, P], BF16),
        ('wfc1', [HM, P, C, P], BF16), ('wfc2', [C, P, HM, P], BF16),
        ('bq', [P, C], F32), ('bk', [P, C], F32),
        ('bcq', [P, C], F32), ('bck', [P, C], F32),
        ('bproj', [P, C], F32), ('bcproj', [P, C], F32),
        ('bfc1', [P, HM], F32), ('bfc2', [P, C], F32),
        ('bv_row', [1, DIM], BF16), ('bcv_row', [1, DIM], BF16),
        ('ones_bf', [P, P], BF16), ('prot', [P, P], BF16),
    ]:
        d[name] = nc.declare_dram_parameter(name, shape, dt, isOutput=False)
    out_d = nc.declare_dram_parameter('outT', [DIM, N], BF16, isOutput=True)

    with TileContext(nc) as tc:
        with tc.tile_pool(name="const", bufs=1) as const, \
             tc.tile_pool(name="main", bufs=1) as main, \
             tc.tile_pool(name="work", bufs=2) as work, \
             tc.tile_pool(name="dscr", bufs=4, space="DRAM") as dscr, \
             tc.tile_pool(name="ps", bufs=1, space="PSUM") as psp:

            def body():
                # ---- constants ----
                cos_x = const.tile([P, N], BF16, tag='cosx', name='cos_x')
                sin_x = const.tile([P, N], BF16, tag='sinx', name='sin_x')
                cos_y = const.tile([P, N], BF16, tag='cosy', name='cos_y')
                sin_y = const.tile([P, N], BF16, tag='siny', name='sin_y')
                nc.sync.dma_start(cos_x[:], d['cosx'][:])
                nc.sync.dma_start(sin_x[:], d['sinx'][:])
                nc.sync.dma_start(cos_y[:], d['cosy'][:])
                nc.sync.dma_start(sin_y[:], d['siny'][:])
                ones_bf = const.tile([P, P], BF16, tag='ones', name='ones_bf')
                nc.sync.dma_start(ones_bf[:], d['ones_bf'][:])
                prot = const.tile([P, P], BF16, tag='prot', name='prot')
                nc.sync.dma_start(prot[:], d['prot'][:])
                bias = {}
                for nm in ('bq', 'bk', 'bcq', 'bck', 'bproj', 'bcproj',
                           'bfc2'):
                    bias[nm] = const.tile([P, C], F32, tag=nm, name=nm)
                    nc.sync.dma_start(bias[nm][:], d[nm][:])
                bias['bfc1'] = const.tile([P, HM], F32, tag='bfc1',
                                          name='bfc1')
                nc.sync.dma_start(bias['bfc1'][:], d['bfc1'][:])
                eps_t = const.tile([P, 1], F32, tag='eps', name='eps_t')
                nc.vector.memset(eps_t[:], EPS)
                bvrep = const.tile([P, DIM], BF16, tag='bvrep', name='bvrep')
                nc.sync.dma_start(bvrep[:],
                                  d['bv_row'][:].to_broadcast((P, DIM)))
                bcvrep = const.tile([P, DIM], BF16, tag='bcvrep',
                                    name='bcvrep')
                nc.sync.dma_start(bcvrep[:],
                                  d['bcv_row'][:].to_broadcast((P, DIM)))

                def load_chunks(dst, src_d):
                    for cc in range(C):
                        nc.sync.dma_start(dst[:, cc, :],
                                          src_d[cc * P:(cc + 1) * P, :])

                def w_cols(wd, colb, r0=0, rcnt=None, tag='wlhs', bufs=4):
                    """Pre-tiled weight block [128, rcnt, 128] bf16."""
                    cr = wd.shape[2]
                    if rcnt is None:
                        rcnt = cr
                    t = work.tile([P, rcnt, P], BF16, tag=tag, bufs=bufs,
                                  name=f'w_{tag}')
                    nc.sync.dma_start(t[:], wd[colb, :, r0:r0 + rcnt, :])
                    return t

                def layernorm_units(src, dst_tag, xsq_tag='oTB'):
                    """src [128, C, N] bf16 -> (h, emit(qh)): the stats and
                    apply for each 512-column half are emitted separately so
                    a half can overlap the other half's producer (proj)."""
                    xsq = main.tile([P, C, N], BF16, tag=xsq_tag, name='xsq')
                    m_bf = work.tile([P, N], BF16, tag='m_bf', bufs=1,
                                     name='m_bf')
                    var = work.tile([P, N], F32, tag='var', bufs=1,
                                    name='var')
                    rstd = work.tile([P, N], BF16, tag='rstd', bufs=1,
                                     name='rstd')
                    h = main.tile([P, C, N], BF16, tag=dst_tag, name='h_out')

                    def emit(qh):
                        sl = slice(qh * 512, qh * 512 + 512)
                        for cc in range(C):
                            nc.gpsimd.tensor_tensor(
                                xsq[:, cc, sl], src[:, cc, sl],
                                src[:, cc, sl], OP.mult)
                        ps1 = psp.tile([P, 512], F32, tag='pq', bufs=2,
                                       name='ps_sum')
                        ps2 = psp.tile([P, 512], F32, tag='pq', bufs=2,
                                       name='ps_sumsq')
                        for cc in range(C):
                            nc.tensor.matmul(
                                ps1[:], ones_bf[:], src[:, cc, sl],
                                start=(cc == 0), stop=(cc == C - 1))
                            nc.tensor.matmul(
                                ps2[:], ones_bf[:], xsq[:, cc, sl],
                                start=(cc == 0), stop=(cc == C - 1))
                        nc.vector.tensor_scalar_mul(m_bf[:, sl], ps1[:],
                                                    1.0 / DIM)
                        msq = work.tile([P, 512], BF16, tag='msq', bufs=2,
                                        name='msq')
                        nc.vector.tensor_tensor(msq[:], m_bf[:, sl],
                                                m_bf[:, sl], OP.mult)
                        nc.vector.scalar_tensor_tensor(
                            var[:, sl], ps2[:], 1.0 / DIM, msq[:],
                            OP.mult, OP.subtract)
                        nc.scalar.activation(var[:, sl], var[:, sl], AF.Ln,
                                             bias=eps_t[:])
                        nc.scalar.activation(rstd[:, sl], var[:, sl], AF.Exp,
                                             scale=-0.5)
                        for cc in range(C):
                            tmp = work.tile([P, 512], BF16, tag='lntmp',
                                            bufs=2, name='lntmp')
                            nc.vector.tensor_tensor(
                                tmp[:], src[:, cc, sl], m_bf[:, sl],
                                OP.subtract)
                            nc.vector.tensor_tensor(
                                h[:, cc, sl], tmp[:], rstd[:, sl], OP.mult)

                    return h, emit

                def layernorm(src, dst_tag):
                    h, emit = layernorm_units(src, dst_tag)
                    emit(0)
                    emit(1)
                    return h

                def build_vt_units(h, wv_d, bvr, tag):
                    """V+ones lhsT tile [128, T, HEADS, 128] bf16, emitted
                    as one work unit per token chunk (for interleaving)."""
                    vt = main.tile([P, T, HEADS, P], BF16, tag=tag,
                                   name='vt')
                    state = {}

                    def unit(tci):
                        if 'wvt' not in state:
                            nc.gpsimd.memset(vt[:, :, 0:HEADS:2, 64:65], 1.0)
                            nc.gpsimd.memset(vt[:, :, 1:HEADS:2, 63:64], 1.0)
                            wvt = main.tile([P, C, DIM], BF16, tag='wv_full',
                                            name='wvt')
                            nc.sync.dma_start(wvt[:], wv_d[:])
                            state['wvt'] = wvt
                        wvt = state['wvt']
                        pv = psp.tile([P, N], F32, tag='pss', bufs=2,
                                      name='pv')
                        for cc in range(C):
                            lhs = h[:, cc, tci * P:(tci + 1) * P]
                            nc.tensor.matmul(
                                pv[:, 0:512], lhs, wvt[:, cc, 0:512],
                                start=(cc == 0), stop=(cc == C - 1))
                            nc.tensor.matmul(
                                pv[:, 512:768], lhs, wvt[:, cc, 512:768],
                                start=(cc == 0), stop=(cc == C - 1))
                        pv_h = pv[:, 0:DIM].rearrange("p (h e) -> p h e",
                                                      e=HD)
                        bv_h = bvr[:].rearrange("p (h e) -> p h e", e=HD)
                        nc.vector.tensor_tensor(
                            vt[:, tci, 0:HEADS:2, 0:64],
                            pv_h[:, 0:HEADS:2, :], bv_h[:, 0:HEADS:2, :],
                            OP.add)
                        nc.vector.tensor_tensor(
                            vt[:, tci, 1:HEADS:2, 64:128],
                            pv_h[:, 1:HEADS:2, :], bv_h[:, 1:HEADS:2, :],
                            OP.add)

                    units = [lambda tci=tci: unit(tci) for tci in range(T)]
                    return vt, units

                def build_vt(h, wv_d, bvr, tag):
                    vt, units = build_vt_units(h, wv_d, bvr, tag)
                    for u in units:
                        u()
                    return vt

                def rope_qk(h_src, w_d, b_sb, cost, sint, pi, tag):
                    """Roped Q^T or K^T pair-chunk [128, 1024] bf16."""
                    out = work.tile([P, N], BF16, tag=tag, name=f'{tag}_t')
                    wa = w_cols(w_d, pi)
                    for qh in range(2):
                        sl = slice(qh * 512, qh * 512 + 512)
                        pq = psp.tile([P, 512], F32, tag='pq', bufs=2,
                                      name='pq')
                        for cc in range(C):
                            nc.tensor.matmul(
                                pq[:], wa[:, cc, :], h_src[:, cc, sl],
                                start=(cc == 0), stop=(cc == C - 1))
                        u = work.tile([P, 512], BF16, tag='u', bufs=2,
                                      name='u')
                        nc.vector.tensor_scalar_add(u[:], pq[:],
                                                    b_sb[:, pi:pi + 1])
                        t1 = work.tile([P, 512], BF16, tag='t1', bufs=2,
                                       name='t1')
                        nc.vector.tensor_tensor(t1[:], u[:], cost[:, sl],
                                                OP.mult)
                        # rotate-half via permutation matmul, reusing pq
                        nc.tensor.matmul(pq[:], prot[:], u[:],
                                         start=True, stop=True)
                        t2 = work.tile([P, 512], BF16, tag='t2', bufs=2,
                                       name='t2')
                        nc.vector.tensor_tensor(t2[:], pq[:], sint[:, sl],
                                                OP.mult)
                        nc.vector.tensor_tensor(out[:, sl], t1[:], t2[:],
                                                OP.add)
                    return out

                def attention(h_q, h_k, vt, wq_d, wk_d, bq_sb, bk_sb,
                              cos_q, sin_q, cos_k, sin_k, oT, fillers=()):
                    fill_iter = iter(fillers)

                    def fill(n):
                        for _ in range(n):
                            f = next(fill_iter, None)
                            if f is not None:
                                f()

                    for pi in range(NP):
                        qro = rope_qk(h_q, wq_d, bq_sb, cos_q, sin_q, pi,
                                      'qro')
                        kro = rope_qk(h_k, wk_d, bk_sb, cos_k, sin_k, pi,
                                      'kro')
                        fill(2 if pi >= NP - 2 else 1)
                        for qh in range(2):
                            qsl = slice(qh * 512, qh * 512 + 512)
                            pav = psp.tile([P, N], F32, tag='pav', bufs=1,
                                           name='pav')
                            for kc in range(KC):
                                ksl = slice(kc * P, (kc + 1) * P)
                                pss = psp.tile([P, N], F32, tag='pss',
                                               bufs=2, name='pss')
                                nc.tensor.matmul(
                                    pss[:, 0:512], kro[0:64, ksl],
                                    qro[0:64, qsl], start=True, stop=True)
                                nc.tensor.matmul(
                                    pss[:, 512:1024], kro[64:128, ksl],
                                    qro[64:128, qsl], start=True, stop=True)
                                ex = work.tile([P, 2, 512], BF16, tag='expS',
                                               bufs=4, name='expS')
                                nc.scalar.activation(
                                    ex[:], pss[:], AF.Exp,
                                    scale=float(HD) ** -0.5)
                                nc.tensor.matmul(
                                    pav[:, 0:512], vt[:, kc, 2 * pi, :],
                                    ex[:, 0, :],
                                    start=(kc == 0), stop=(kc == KC - 1))
                                nc.tensor.matmul(
                                    pav[:, 512:1024],
                                    vt[:, kc, 2 * pi + 1, :], ex[:, 1, :],
                                    start=(kc == 0), stop=(kc == KC - 1))
                            # evacuate AV out of PSUM right away (frees the
                            # bank; makes the normalize TTs bf16 2x-mode)
                            pavS = work.tile([P, N], BF16, tag='pavS',
                                             bufs=2, name='pavS')
                            nc.vector.tensor_copy(pavS[:], pav[:])
                            # denominators: row 64 (even head), row 63 (odd)
                            tln = work.tile([P, N], F32, tag='tln', bufs=1,
                                            name='tln')
                            nc.scalar.activation(tln[:], pav[:], AF.Ln)
                            trec = work.tile([P, N], BF16, tag='trec',
                                             bufs=1, name='trec')
                            nc.scalar.activation(trec[:], tln[:], AF.Exp,
                                                 scale=-1.0)
                            scr = dscr.tile([2, N], BF16, tag='scr',
                                            name='scr')
                            nc.sync.dma_start(scr[:], trec[63:65, :])
                            rep = work.tile([P, N], BF16, tag='rep', bufs=2,
                                            name='rep')
                            nc.sync.dma_start(
                                rep[0:64, 0:512],
                                scr[1:2, 0:512].to_broadcast((64, 512)))
                            nc.sync.dma_start(
                                rep[64:128, 512:1024],
                                scr[0:1, 512:1024].to_broadcast((64, 512)))
                            nc.vector.tensor_tensor(
                                oT[0:64, pi, qsl], pavS[0:64, 0:512],
                                rep[0:64, 0:512], OP.mult)
                            nc.vector.tensor_tensor(
                                oT[64:128, pi, qsl], pavS[64:128, 512:1024],
                                rep[64:128, 512:1024], OP.mult)
                    fill(T)

                def proj_qh(oT, w_d, b_sb, qh):
                    sl = slice(qh * 512, qh * 512 + 512)
                    for m in range(C):
                        pp = psp.tile([P, 512], F32, tag='pq', bufs=2,
                                      name='pp')
                        wp = w_cols(w_d, m)
                        for cc in range(C):
                            nc.tensor.matmul(
                                pp[:], wp[:, cc, :], oT[:, cc, sl],
                                start=(cc == 0), stop=(cc == C - 1))
                        nc.vector.scalar_tensor_tensor(
                            xT[:, m, sl], pp[:], b_sb[:, m:m + 1],
                            xT[:, m, sl], OP.add, OP.add)

                def proj_then_ln(oT, w_d, b_sb, dst_tag):
                    """Residual projection + next LayerNorm, column-half
                    interleaved: half-0's LN chain (DVE/ACT) overlaps
                    half-1's projection matmuls (PE)."""
                    # xsq must not alias oT ('oTB') here: its half-0 write
                    # would WAR-wait half-1 proj reads emitted later. yT is
                    # long dead by the first transition.
                    h, emit = layernorm_units(xT, dst_tag, xsq_tag='yT')
                    for qh in range(2):
                        proj_qh(oT, w_d, b_sb, qh)
                        emit(qh)
                    return h

                # ---- y side first (fills the x-load head) ----
                yT = main.tile([P, C, N], BF16, tag='yT', name='yT')
                load_chunks(yT, d['yT'])
                y_ = layernorm(yT, 'hy')

                # ---- x residual load ----
                xT = main.tile([P, C, N], BF16, tag='xT', name='xT')
                load_chunks(xT, d['xT'])

                # ================= self attention =================
                h1 = layernorm(xT, 'h')
                vt = build_vt(h1, d['wv'], bvrep, 'bigA')
                # cross-attn V build interleaves into self-attn's
                # ScalarE-bound pair loop (it only depends on y)
                vtc, vtc_units = build_vt_units(y_, d['wcv'], bcvrep, 'hidh')
                oT1 = main.tile([P, C, N], BF16, tag='oTB', name='oT1')
                attention(h1, h1, vt, d['wqA'], d['wkA'],
                          bias['bq'], bias['bk'],
                          cos_x, sin_x, cos_x, sin_x, oT1,
                          fillers=vtc_units)
                h2 = proj_then_ln(oT1, d['wproj'], bias['bproj'], 'h')

                # ================= cross attention =================
                oT2 = main.tile([P, C, N], BF16, tag='oTB', name='oT2')
                attention(h2, y_, vtc, d['wcqA'], d['wckA'],
                          bias['bcq'], bias['bck'],
                          cos_x, sin_x, cos_y, sin_y, oT2)
                h3 = proj_then_ln(oT2, d['wcproj'], bias['bcproj'], 'h')

                # ================= MLP (two hidden halves) =================
                HH = HM // 2
                outacc = main.tile([P, C, N], F32, tag='bigA', name='outacc')
                for half in range(2):
                    hid = main.tile([P, HH, N], BF16, tag='hidh', name='hid')
                    for hj in range(HH):
                        hm = half * HH + hj
                        ph = psp.tile([P, N], F32, tag='pss', bufs=2,
                                      name='ph')
                        wf = w_cols(d['wfc1'], hm)
                        for cc in range(C):
                            for qh in range(2):
                                sl = slice(qh * 512, qh * 512 + 512)
                                nc.tensor.matmul(
                                    ph[:, sl], wf[:, cc, :], h3[:, cc, sl],
                                    start=(cc == 0), stop=(cc == C - 1))
                        nc.scalar.activation(
                            hid[:, hj, :], ph[:], AF.Gelu,
                            bias=bias['bfc1'][:, hm:hm + 1])
                    for m in range(C):
                        po = psp.tile([P, N], F32, tag='pss', bufs=2,
                                      name='po')
                        wf2 = w_cols(d['wfc2'], m, half * HH, HH,
                                     tag='wlhs2', bufs=2)
                        for kj in range(HH):
                            for qh in range(2):
                                sl = slice(qh * 512, qh * 512 + 512)
                                nc.tensor.matmul(
                                    po[:, sl], wf2[:, kj, :],
                                    hid[:, kj, sl],
                                    start=(kj == 0), stop=(kj == HH - 1))
                        if half == 0:
                            nc.vector.tensor_scalar_add(
                                outacc[:, m, :], po[:],
                                bias['bfc2'][:, m:m + 1])
                        else:
                            nc.vector.tensor_tensor(
                                outacc[:, m, :], outacc[:, m, :], po[:],
                                OP.add)
                            nc.vector.tensor_tensor(
                                xT[:, m, :], xT[:, m, :], outacc[:, m, :],
                                OP.add)
                            nc.sync.dma_start(out_d[m * P:(m + 1) * P, :],
                                              xT[:, m, :])

            if k_iters > 1:
                with tc.For_i(0, k_iters, 1):
                    body()
            else:
                body()

    split_excess_waits(nc)
    return nc


# ------------------------------------------------------------------ driver

def kernel(**inputs):
    from concourse.bass_utils import run_bass_kernel_spmd
    per_core = prep_host(inputs)
    nc = build_nc(1)
    res = run_bass_kernel_spmd(nc, per_core, core_ids=list(range(B)))
    x_out = np.stack([np.ascontiguousarray(
        res.results[i]['outT'].astype(np.float32).T) for i in range(B)])
    y = np.asarray(inputs['y'], dtype=np.float32)
    return (x_out.astype(np.float32), y)
